# revision 6
# baseline (speedup 1.0000x reference)
"""Trainium2 Bass kernel for nn_AttnLayer (additive-attention pooling layer).

Reference computation (per batch b):
    e = e_hiddens @ We_w.T + We_b            # [S, F]
    d = Wd_w @ d_hiddens[b]                  # [F]
    h = tanh(d + e)                          # [S, F]
    s = h @ v_w[0] + v_b                     # [S]
    a = softmax(s)                           # [S]
    out[b] = a @ e_hiddens[b]                # [D]

Strategy (8 cores, data-parallel over batch B=32 -> 4 per core):
  Single pass over e_hiddens (the only big tensor, 64 MB/core).  Natural
  [s, d] tiles are DMA'd once; PE transposes them (float32r) to get
  d-partition tiles for the We matmul; scores are computed in [f, s]
  orientation (tanh bias = per-partition decoder vector); exp is applied
  without max-subtraction (tanh bounds |score| <= sum|v| + |b|, no
  overflow in fp32); the weighted sum over S runs off the same
  natural-layout tiles with the exp-weights as the stationary operand,
  accumulating unnormalized in PSUM; one divide by Z at batch end.

All tensor-engine operands are float32r (fp32 storage, single-pass
reduced-precision matmul, 4x faster than strict fp32 on trn2).
"""

import numpy as np

import concourse.bass as bass
import concourse.bacc as bacc
import concourse.mybir as mybir
import concourse.tile as tile
from concourse.bass_utils import run_bass_kernel_spmd

F32 = mybir.dt.float32
F32R = mybir.dt.float32r
AF = mybir.ActivationFunctionType

N_CORES = 8
B, S, D, F = 32, 4096, 1024, 128
BP = B // N_CORES  # batches per core

S_SUB = 128        # s-subchunk (partition dim of natural x tiles)
S_CHUNK = 512      # s-chunk (free dim of matmul1 rhs)


def f32(ap):
    return ap.bitcast(F32)


def build_nc(bp=BP, s=S, d=D, f=F, x_bufs=12):
    kd = d // 128
    nsub = S_CHUNK // S_SUB
    nchunk = s // S_CHUNK
    d_halves = [(lo, min(lo + 512, d)) for lo in range(0, d, 512)]

    nc = bacc.Bacc("TRN2", target_bir_lowering=False, debug=False)

    x_dram = nc.dram_tensor("x", [bp, s, d], F32R, kind="ExternalInput").ap()
    wet_dram = nc.dram_tensor("wet", [d, f], F32R, kind="ExternalInput").ap()
    web_dram = nc.dram_tensor("web", [f, 1], F32, kind="ExternalInput").ap()
    wdt_dram = nc.dram_tensor("wdt", [d, f], F32R, kind="ExternalInput").ap()
    dht_dram = nc.dram_tensor("dht", [d, bp], F32R, kind="ExternalInput").ap()
    vcol_dram = nc.dram_tensor("vcol", [f, 2], F32R, kind="ExternalInput").ap()
    vbb_dram = nc.dram_tensor("vbb", [S_SUB, 1], F32, kind="ExternalInput").ap()
    ident_dram = nc.dram_tensor("ident", [128, 128], F32R, kind="ExternalInput").ap()
    ones_dram = nc.dram_tensor("ones", [S_SUB, 2], F32R, kind="ExternalInput").ap()
    out_dram = nc.dram_tensor("out", [bp, d], F32, kind="ExternalOutput").ap()

    with tile.TileContext(nc) as tc:
        with (
            tc.tile_pool(name="const", bufs=1) as const,
            tc.tile_pool(name="xpool", bufs=x_bufs) as xpool,
            tc.tile_pool(name="xtpool", bufs=2) as xtpool,
            tc.tile_pool(name="hpool", bufs=2) as hpool,
            tc.tile_pool(name="wpool", bufs=2) as wpool,
            tc.tile_pool(name="opool", bufs=2) as opool,
            tc.tile_pool(name="ps_xt", bufs=1, space="PSUM") as ps_xt,
            tc.tile_pool(name="ps_e", bufs=1, space="PSUM") as ps_e,
            tc.tile_pool(name="ps_sc", bufs=1, space="PSUM") as ps_sc,
            tc.tile_pool(name="ps_wa", bufs=1, space="PSUM") as ps_wa,
            tc.tile_pool(name="ps_z", bufs=1, space="PSUM") as ps_z,
            tc.tile_pool(name="ps_dv", bufs=1, space="PSUM") as ps_dv,
        ):
            # ---- constants ----
            ident = const.tile([128, 128], F32R)
            nc.sync.dma_start(ident, ident_dram)
            wet_sb = const.tile([128, kd, f], F32R)
            nc.sync.dma_start(wet_sb, wet_dram.rearrange("(k p) f -> p k f", p=128))
            web_sb = const.tile([f, 1], F32)
            nc.sync.dma_start(web_sb, web_dram)
            wdt_sb = const.tile([128, kd, f], F32R)
            nc.sync.dma_start(wdt_sb, wdt_dram.rearrange("(k p) f -> p k f", p=128))
            dht_sb = const.tile([128, kd, bp], F32R)
            nc.sync.dma_start(dht_sb, dht_dram.rearrange("(k p) b -> p k b", p=128))
            vcol_sb = const.tile([f, 2], F32R)
            nc.sync.dma_start(vcol_sb, vcol_dram)
            vbb_sb = const.tile([S_SUB, 1], F32)
            nc.sync.dma_start(vbb_sb, vbb_dram)
            ones_sb = const.tile([S_SUB, 2], F32R)
            nc.sync.dma_start(ones_sb, ones_dram)

            # ---- decoder projection: dvec[f, b] = Wd @ d_hiddens.T ----
            dv_ps = ps_dv.tile([f, bp], F32)
            for k in range(kd):
                nc.tensor.matmul(
                    dv_ps, wdt_sb[:, k, :], dht_sb[:, k, :],
                    start=(k == 0), stop=(k == kd - 1),
                )
            # fold the We bias (varies along f, broadcast over s) into dvec
            dvec_sb = const.tile([f, bp], F32)
            nc.vector.tensor_scalar_add(dvec_sb, dv_ps, web_sb)

            # ---- main loop ----
            for b in range(bp):
                wacc_ps = ps_wa.tile([2, d], F32, tag="wacc")
                z_ps = ps_z.tile([2, 2], F32, tag="z")
                for c in range(nchunk):
                    # load natural tiles [s_sub, d] (+ ones column at d)
                    x_t = []
                    for j in range(nsub):
                        s0 = c * S_CHUNK + j * S_SUB
                        xt = xpool.tile([S_SUB, d], F32R, tag="x", name=f"x_{b}_{c}_{j}")
                        nc.sync.dma_start(xt, x_dram[b, s0:s0 + S_SUB, :])
                        x_t.append(xt)

                    # transpose to [d_part, s] layout for the We matmul
                    xt_sb = xtpool.tile([128, kd, S_CHUNK], F32R, tag="xt", name=f"xt_{b}_{c}")
                    groups_per_bank = max(1, 2048 // (S_SUB * 4))
                    for j in range(nsub):
                        xt_ps = ps_xt.tile([128, kd * S_SUB], F32R, tag="xtp", name=f"xtp_{b}_{c}_{j}")
                        for k in range(kd):
                            gi = k % groups_per_bank
                            nc.tensor.matmul(
                                xt_ps[:, k * S_SUB:(k + 1) * S_SUB],
                                x_t[j][:, k * 128:(k + 1) * 128],
                                ident,
                                is_transpose=True,
                                start=(gi == 0),
                                stop=(gi == groups_per_bank - 1 or k == kd - 1),
                            )
                        xt_ps_v = xt_ps.rearrange("p (k s) -> p k s", k=kd)
                        xt_dst = xt_sb[:, :, j * S_SUB:(j + 1) * S_SUB]
                        if j % 2 == 0:
                            nc.vector.tensor_copy(xt_dst, xt_ps_v)
                        else:
                            nc.scalar.copy(xt_dst, xt_ps_v)

                    # e^T[f, s] = We @ x^T  (accumulate over kd)
                    e_ps = ps_e.tile([f, S_CHUNK], F32, tag="e", name=f"e_{b}_{c}")
                    for k in range(kd):
                        nc.tensor.matmul(
                            e_ps, wet_sb[:, k, :], xt_sb[:, k, :],
                            start=(k == 0), stop=(k == kd - 1),
                        )

                    # h = tanh(e + dvec[:, b])  (bias is per-partition)
                    h_sb = hpool.tile([f, S_CHUNK], F32R, tag="h", name=f"h_{b}_{c}")
                    nc.scalar.activation(h_sb, e_ps, AF.Tanh, bias=dvec_sb[:, b:b + 1])

                    # scores[s_sub, j] = h_tile.T @ v   (one psum group, 4 cols)
                    sc_ps = ps_sc.tile([S_SUB, 2 * nsub], F32, tag="sc", name=f"sc_{b}_{c}")
                    for j in range(nsub):
                        nc.tensor.matmul(
                            sc_ps[:, 2 * j:2 * j + 2],
                            h_sb[:, j * S_SUB:(j + 1) * S_SUB],
                            vcol_sb,
                            start=(j == 0), stop=(j == nsub - 1),
                        )

                    # w = exp(scores + v_b)
                    wm_sb = wpool.tile([S_SUB, 2 * nsub], F32R, tag="wm", name=f"wm_{b}_{c}")
                    nc.scalar.activation(wm_sb, sc_ps, AF.Exp, bias=vbb_sb)

                    # unnormalized weighted sum + Z accumulation
                    first = c == 0
                    last = c == nchunk - 1
                    for j in range(nsub):
                        wcol = wm_sb[:, 2 * j:2 * j + 2]
                        for lo, hi in d_halves:
                            nc.tensor.matmul(
                                wacc_ps[:, lo:hi], wcol, x_t[j][:, lo:hi],
                                start=(first and j == 0), stop=(last and j == nsub - 1),
                            )
                        nc.tensor.matmul(
                            z_ps, wcol, ones_sb,
                            start=(first and j == 0), stop=(last and j == nsub - 1),
                        )

                # normalize and store
                zr_sb = opool.tile([1, 1], F32, tag="zr", name=f"zr_{b}")
                nc.vector.reciprocal(zr_sb, z_ps[0:1, 0:1])
                out_sb = opool.tile([1, d], F32, tag="o", name=f"o_{b}")
                nc.vector.tensor_scalar_mul(out_sb, wacc_ps[0:1, :], zr_sb)
                nc.sync.dma_start(out_dram[b:b + 1, :], out_sb)

    nc.finalize()
    return nc


_NC_CACHE = {}


def _get_nc(key, **kw):
    if key not in _NC_CACHE:
        _NC_CACHE[key] = build_nc(**kw)
    return _NC_CACHE[key]


def make_in_maps(e_hiddens, d_hiddens, We_w, We_b, Wd_w, v_w, v_b, n_cores=N_CORES):
    bp = e_hiddens.shape[0] // n_cores
    wet = np.ascontiguousarray(We_w.T)          # [D, F]
    web = np.ascontiguousarray(We_b[:, None])   # [F, 1]
    wdt = np.ascontiguousarray(Wd_w.T)          # [D, F]
    vcol = np.ascontiguousarray(np.repeat(v_w[0][:, None], 2, axis=1))  # [F, 2]
    vbb = np.full((S_SUB, 1), np.float32(v_b[0]), np.float32)
    ident = np.eye(128, dtype=np.float32)
    maps = []
    for i in range(n_cores):
        maps.append({
            "x": np.ascontiguousarray(e_hiddens[i * bp:(i + 1) * bp]),
            "wet": wet,
            "web": web,
            "wdt": wdt,
            "dht": np.ascontiguousarray(d_hiddens[i * bp:(i + 1) * bp].T),
            "vcol": vcol,
            "vbb": vbb,
            "ident": ident,
            "ones": np.ones((S_SUB, 2), np.float32),
        })
    return maps


def kernel(e_hiddens, d_hiddens, length_mask, We_w, We_b, Wd_w, v_w, v_b,
           _trace=False):
    """Full inputs in, full output out.  length_mask is all-ones (the
    reference adds (1-mask)*1e-32, numerically a no-op)."""
    e_hiddens = np.asarray(e_hiddens, np.float32)
    d_hiddens = np.asarray(d_hiddens, np.float32)
    We_w = np.asarray(We_w, np.float32)
    We_b = np.asarray(We_b, np.float32)
    Wd_w = np.asarray(Wd_w, np.float32)
    v_w = np.asarray(v_w, np.float32)
    v_b = np.asarray(v_b, np.float32)

    nc = _get_nc("full")
    in_maps = make_in_maps(e_hiddens, d_hiddens, We_w, We_b, Wd_w, v_w, v_b)
    res = run_bass_kernel_spmd(nc, in_maps, list(range(N_CORES)), trace=_trace)
    out = np.concatenate([m["out"] for m in res.results], axis=0)
    if _trace:
        kernel.last_results = res
    return out


# revision 10
# speedup vs baseline: 45264.9604x; 45264.9604x over previous
"""Trainium2 Bass kernel for nn_AttnLayer (additive-attention pooling layer).

Reference computation (per batch b):
    e = e_hiddens @ We_w.T + We_b            # [S, F]
    d = Wd_w @ d_hiddens[b]                  # [F]
    h = tanh(d + e)                          # [S, F]
    s = h @ v_w[0] + v_b                     # [S]
    a = softmax(s)                           # [S]
    out[b] = a @ e_hiddens[b]                # [D]

Strategy (8 cores, data-parallel over batch B=32 -> 4 per core):
  Single pass over e_hiddens (the only big tensor, 64 MB/core).  Natural
  [s, d] tiles are DMA'd once; PE transposes them (float32r) to get
  d-partition tiles for the We matmul; scores are computed in [f, s]
  orientation (tanh bias = per-partition decoder vector); exp is applied
  without max-subtraction (tanh bounds |score| <= sum|v| + |b|, no
  overflow in fp32); the weighted sum over S runs off the same
  natural-layout tiles with the exp-weights as the stationary operand,
  accumulating unnormalized in PSUM; one divide by Z at batch end.

All tensor-engine operands are float32r (fp32 storage, single-pass
reduced-precision matmul, 4x faster than strict fp32 on trn2).
"""

import numpy as np

import concourse.bass as bass
import concourse.bacc as bacc
import concourse.mybir as mybir
import concourse.tile as tile
from concourse.bass_utils import run_bass_kernel_spmd

F32 = mybir.dt.float32
F32R = mybir.dt.float32r
AF = mybir.ActivationFunctionType

N_CORES = 8
B, S, D, F = 32, 4096, 1024, 128
BP = B // N_CORES  # batches per core

S_SUB = 128        # s-subchunk (partition dim of natural x tiles)
S_CHUNK = 512      # s-chunk (free dim of matmul1 rhs)


def f32(ap):
    return ap.bitcast(F32)


def build_nc(bp=BP, s=S, d=D, f=F, x_bufs=10):
    kd = d // 128
    nsub = S_CHUNK // S_SUB
    nchunk = s // S_CHUNK
    d_halves = [(lo, min(lo + 512, d)) for lo in range(0, d, 512)]

    nc = bacc.Bacc("TRN2", target_bir_lowering=False, debug=False)

    x_dram = nc.dram_tensor("x", [bp, s, d], F32R, kind="ExternalInput").ap()
    wet_dram = nc.dram_tensor("wet", [d, f], F32R, kind="ExternalInput").ap()
    web_dram = nc.dram_tensor("web", [f, 1], F32, kind="ExternalInput").ap()
    wdt_dram = nc.dram_tensor("wdt", [d, f], F32R, kind="ExternalInput").ap()
    dht_dram = nc.dram_tensor("dht", [d, bp], F32R, kind="ExternalInput").ap()
    vcol_dram = nc.dram_tensor("vcol", [f, 2], F32R, kind="ExternalInput").ap()
    vbb_dram = nc.dram_tensor("vbb", [S_SUB, 1], F32, kind="ExternalInput").ap()
    ident_dram = nc.dram_tensor("ident", [128, 128], F32R, kind="ExternalInput").ap()
    ones_dram = nc.dram_tensor("ones", [S_SUB, 2], F32R, kind="ExternalInput").ap()
    out_dram = nc.dram_tensor("out", [bp, d], F32, kind="ExternalOutput").ap()

    with tile.TileContext(nc) as tc:
        with (
            tc.tile_pool(name="const", bufs=1) as const,
            tc.tile_pool(name="xpool", bufs=x_bufs) as xpool,
            tc.tile_pool(name="xtpool", bufs=3) as xtpool,
            tc.tile_pool(name="hpool", bufs=3) as hpool,
            tc.tile_pool(name="wpool", bufs=2) as wpool,
            tc.tile_pool(name="opool", bufs=2) as opool,
            tc.tile_pool(name="ps_xt", bufs=2, space="PSUM") as ps_xt,
            tc.tile_pool(name="ps_e", bufs=1, space="PSUM") as ps_e,
            tc.tile_pool(name="ps_sc", bufs=1, space="PSUM") as ps_sc,
            tc.tile_pool(name="ps_wa", bufs=1, space="PSUM") as ps_wa,
        ):
            # ---- constants ----
            ident = const.tile([128, 128], F32R)
            nc.sync.dma_start(ident, ident_dram)
            wet_sb = const.tile([128, kd, f], F32R)
            nc.sync.dma_start(wet_sb, wet_dram.rearrange("(k p) f -> p k f", p=128))
            web_sb = const.tile([f, 1], F32)
            nc.sync.dma_start(web_sb, web_dram)
            wdt_sb = const.tile([128, kd, f], F32R)
            nc.sync.dma_start(wdt_sb, wdt_dram.rearrange("(k p) f -> p k f", p=128))
            dht_sb = const.tile([128, kd, bp], F32R)
            nc.sync.dma_start(dht_sb, dht_dram.rearrange("(k p) b -> p k b", p=128))
            vcol_sb = const.tile([f, 2], F32R)
            nc.sync.dma_start(vcol_sb, vcol_dram)
            vbb_sb = const.tile([S_SUB, 1], F32)
            nc.sync.dma_start(vbb_sb, vbb_dram)
            ones_sb = const.tile([S_SUB, 2], F32R)
            nc.sync.dma_start(ones_sb, ones_dram)

            # ---- decoder projection: dvec[f, b] = Wd @ d_hiddens.T ----
            dv_ps = ps_sc.tile([f, bp], F32, tag="sc", name="dv_ps")
            for k in range(kd):
                nc.tensor.matmul(
                    dv_ps, wdt_sb[:, k, :], dht_sb[:, k, :],
                    start=(k == 0), stop=(k == kd - 1),
                )
            # fold the We bias (varies along f, broadcast over s) into dvec
            dvec_sb = const.tile([f, bp], F32)
            nc.vector.tensor_scalar_add(dvec_sb, dv_ps, web_sb)

            # ---- main loop, software-pipelined over (batch, chunk) records ----
            # Stage A (record i):   load + transpose + We-matmul + tanh
            # Stage B (record i-1): scores matmul + exp
            # Stage C (record i-2): weighted-sum accumulation (+ batch close)
            # The lag keeps the in-order PE queue from stalling on the
            # ACT-engine tanh/exp results of the current chunk.
            records = [(b, c) for b in range(bp) for c in range(nchunk)]
            wacc_by_b = {}
            zmat_by_b = {}
            state = {}
            groups_per_bank = max(1, 2048 // (S_SUB * 4))

            def stage_a(i):
                b, c = records[i]
                if c == 0:
                    wacc_by_b[b] = ps_wa.tile([2, d], F32, tag="wacc", name=f"wacc_{b}")
                    zmat_by_b[b] = wpool.tile([S_SUB, nchunk], F32, tag="zmat",
                                              name=f"zmat_{b}")
                x_t = []
                for jj in range(nsub // 2):
                    s0 = c * S_CHUNK + jj * 2 * S_SUB
                    x2 = xpool.tile([S_SUB, 2, d], F32R, tag="x", name=f"x_{b}_{c}_{jj}")
                    nc.sync.dma_start(
                        x2, x_dram[b, s0:s0 + 2 * S_SUB, :].rearrange(
                            "(o p) d -> p o d", p=S_SUB))
                    x_t.extend([x2[:, 0, :], x2[:, 1, :]])

                xt_sb = xtpool.tile([128, kd, S_CHUNK], F32R, tag="xt", name=f"xt_{b}_{c}")
                for j in range(nsub):
                    xt_ps = ps_xt.tile([128, kd * S_SUB], F32R, tag="xtp",
                                       name=f"xtp_{b}_{c}_{j}")
                    for k in range(kd):
                        gi = k % groups_per_bank
                        nc.tensor.matmul(
                            xt_ps[:, k * S_SUB:(k + 1) * S_SUB],
                            x_t[j][:, k * 128:(k + 1) * 128],
                            ident,
                            is_transpose=True,
                            start=(gi == 0),
                            stop=(gi == groups_per_bank - 1 or k == kd - 1),
                        )
                    xt_ps_v = xt_ps.rearrange("p (k s) -> p k s", k=kd)
                    xt_dst = xt_sb[:, :, j * S_SUB:(j + 1) * S_SUB]
                    if j % 2 == 0:
                        nc.vector.tensor_copy(xt_dst, xt_ps_v)
                    else:
                        nc.scalar.copy(xt_dst, xt_ps_v)

                e_ps = ps_e.tile([f, S_CHUNK], F32, tag="e", name=f"e_{b}_{c}")
                for k in range(kd):
                    nc.tensor.matmul(
                        e_ps, wet_sb[:, k, :], xt_sb[:, k, :],
                        start=(k == 0), stop=(k == kd - 1),
                    )
                h_sb = hpool.tile([f, S_CHUNK], F32R, tag="h", name=f"h_{b}_{c}")
                nc.scalar.activation(h_sb, e_ps, AF.Tanh, bias=dvec_sb[:, b:b + 1])
                state[i] = {"x_t": x_t, "h": h_sb}

            def stage_b(i):
                b, c = records[i]
                st = state[i]
                sc_ps = ps_sc.tile([S_SUB, 2 * nsub], F32, tag="sc", name=f"sc_{b}_{c}")
                for j in range(nsub):
                    nc.tensor.matmul(
                        sc_ps[:, 2 * j:2 * j + 2],
                        st["h"][:, j * S_SUB:(j + 1) * S_SUB],
                        vcol_sb,
                        start=(j == 0), stop=(j == nsub - 1),
                    )
                wm_sb = wpool.tile([S_SUB, 2 * nsub], F32R, tag="wm", name=f"wm_{b}_{c}", bufs=3)
                nc.scalar.activation(wm_sb, sc_ps, AF.Exp, bias=vbb_sb,
                                     accum_out=zmat_by_b[b][:, c:c + 1])
                st["wm"] = wm_sb

            def stage_c(i):
                b, c = records[i]
                st = state.pop(i)
                wacc_ps = wacc_by_b[b]
                first = c == 0
                last = c == nchunk - 1
                for j in range(nsub):
                    wcol = st["wm"][:, 2 * j:2 * j + 2]
                    for lo, hi in d_halves:
                        nc.tensor.matmul(
                            wacc_ps[:, lo:hi], wcol, st["x_t"][j][:, lo:hi],
                            start=(first and j == 0), stop=(last and j == nsub - 1),
                        )
                if not last:
                    return
                # batch close: copy the accumulator out of PSUM right away so
                # the next batch's weighted-sum (WAR on the psum pool slot)
                # never waits on the normalize chain below.
                wsum_sb = opool.tile([1, d], F32, tag="ws", name=f"ws_{b}")
                nc.scalar.copy(wsum_sb, wacc_by_b.pop(b)[0:1, :])
                # normalize and store.  zmat columns hold per-partition sums
                # of the duplicated-score exp matrix, so Z = sum(zmat) / 2.
                zmat_sb = zmat_by_b.pop(b)
                zmr_sb = wpool.tile([S_SUB, nchunk], F32R, tag="zmr", name=f"zmr_{b}")
                nc.vector.tensor_copy(zmr_sb, zmat_sb)
                zps = ps_sc.tile([2, nchunk], F32, tag="sc", name=f"zps_{b}")
                nc.tensor.matmul(zps, ones_sb, zmr_sb, start=True, stop=True)
                zsum_sb = opool.tile([1, 1], F32, tag="zs", name=f"zs_{b}")
                nc.vector.tensor_reduce(
                    zsum_sb, zps[0:1, :], axis=mybir.AxisListType.X,
                    op=mybir.AluOpType.add)
                zr_sb = opool.tile([1, 1], F32, tag="zr", name=f"zr_{b}")
                nc.vector.reciprocal(zr_sb, zsum_sb)
                out_sb = opool.tile([1, d], F32, tag="o", name=f"o_{b}")
                nc.vector.tensor_scalar(
                    out_sb, wsum_sb, zr_sb, 2.0,
                    op0=mybir.AluOpType.mult, op1=mybir.AluOpType.mult)
                nc.sync.dma_start(out_dram[b:b + 1, :], out_sb)

            n = len(records)
            for i in range(n + 2):
                if i < n:
                    stage_a(i)
                if 1 <= i and i - 1 < n:
                    stage_b(i - 1)
                if 2 <= i:
                    stage_c(i - 2)

    nc.finalize()
    return nc


_NC_CACHE = {}


def _get_nc(key, **kw):
    if key not in _NC_CACHE:
        _NC_CACHE[key] = build_nc(**kw)
    return _NC_CACHE[key]


def make_in_maps(e_hiddens, d_hiddens, We_w, We_b, Wd_w, v_w, v_b, n_cores=N_CORES):
    bp = e_hiddens.shape[0] // n_cores
    wet = np.ascontiguousarray(We_w.T)          # [D, F]
    web = np.ascontiguousarray(We_b[:, None])   # [F, 1]
    wdt = np.ascontiguousarray(Wd_w.T)          # [D, F]
    vcol = np.ascontiguousarray(np.repeat(v_w[0][:, None], 2, axis=1))  # [F, 2]
    vbb = np.full((S_SUB, 1), np.float32(v_b[0]), np.float32)
    ident = np.eye(128, dtype=np.float32)
    maps = []
    for i in range(n_cores):
        maps.append({
            "x": np.ascontiguousarray(e_hiddens[i * bp:(i + 1) * bp]),
            "wet": wet,
            "web": web,
            "wdt": wdt,
            "dht": np.ascontiguousarray(d_hiddens[i * bp:(i + 1) * bp].T),
            "vcol": vcol,
            "vbb": vbb,
            "ident": ident,
            "ones": np.ones((S_SUB, 2), np.float32),
        })
    return maps


def kernel(e_hiddens, d_hiddens, length_mask, We_w, We_b, Wd_w, v_w, v_b,
           _trace=False):
    """Full inputs in, full output out.  length_mask is all-ones (the
    reference adds (1-mask)*1e-32, numerically a no-op)."""
    e_hiddens = np.asarray(e_hiddens, np.float32)
    d_hiddens = np.asarray(d_hiddens, np.float32)
    We_w = np.asarray(We_w, np.float32)
    We_b = np.asarray(We_b, np.float32)
    Wd_w = np.asarray(Wd_w, np.float32)
    v_w = np.asarray(v_w, np.float32)
    v_b = np.asarray(v_b, np.float32)

    nc = _get_nc("full")
    in_maps = make_in_maps(e_hiddens, d_hiddens, We_w, We_b, Wd_w, v_w, v_b)
    res = run_bass_kernel_spmd(nc, in_maps, list(range(N_CORES)), trace=_trace)
    out = np.concatenate([m["out"] for m in res.results], axis=0)
    if _trace:
        kernel.last_results = res
    return out
